# revision 37
# baseline (speedup 1.0000x reference)
"""Block-circulant linear layer (y = x @ W^T + bias, W built from 64x64
circulant blocks) on 8 Trainium2 NeuronCores.

Math: per output block j, input block i: y[t,j] = sum_i circ(c[j,i]) @ x[t,i].
Via the convolution theorem this is, for each rfft bin k:
    Yhat[t,j,k] = sum_i Chat[j,i,k] * Xhat[t,i,k]   (complex)
i.e. 33 independent complex [64 x 64] matmuls over the block index, batched
over tokens. The host does the cheap O(T*F*logB) DFTs + layout packing; the
device does the per-frequency real-packed matmuls.

Real/complex packing (per frequency k, contraction over rows r):
    rhs rows r:   [Xr_i (64) ; Xi_i (64)],  cols = tokens
    lhsT[i,    j] =  Cr[j,i]    lhsT[i,    64+j] = Ci[j,i]
    lhsT[64+i, j] = -Ci[j,i]    lhsT[64+i, 64+j] = Cr[j,i]
    out rows:     [Yr_j (64) ; Yi_j (64)]
Bins k=0 and k=32 are purely real (real input DFT), so they share one tile
(kt=0) with a block-diagonal lhsT; kt=1..31 carry bin k = kt.

Sharding: FREQUENCY-sharded — core m owns kt tiles 4m..4m+3 for ALL 4096
tokens. Same x/y traffic as token-sharding but the lhsT weights are not
replicated (128 KB/core instead of 1 MB/core).

Precision: x is quantized host-side to int8 with a per-(kt, token) scale
(columns of the rhs), shipped as int8 over HBM (halves input DMA bytes) and
cast to fp16 by the SWDGE DMA on load. The matmul is fp16 x fp16 with exact
integer-valued rhs products accumulated in fp32 PSUM, so the device result
equals the host-side integer simulation exactly; output is rounded to bf16
and the host multiplies the scales back in. Measured end-to-end rel err
~6.8e-3 (gate: 2e-2).
"""

import numpy as np
import ml_dtypes

_B = 64          # circulant block size
_NBLK = 64       # input/output blocks (4096/64)
_NK = 33         # rfft bins of a 64-point real signal
_NKT = 32        # packed frequency tiles (k0+k32 share tile 0)
_NCORES = 8
_KTC = _NKT // _NCORES   # kt tiles per core (4)
_T = 4096        # tokens = 2*2048
_F = 4096

_CACHE = {}


def _build_cmat(c):
    """c: [J=64, I=64, B=64] float32 -> packed lhsT matrix [128, NKT*128] fp16."""
    fc = np.fft.rfft(np.asarray(c, np.float32), axis=-1)  # [J, I, 33] complex64
    Cr, Ci = fc.real, fc.imag
    cm = np.zeros((_NKT, 128, 128), np.float32)  # [kt, row, col]
    cm[0, 0:64, 0:64] = Cr[:, :, 0].T
    cm[0, 64:128, 64:128] = Cr[:, :, 32].T
    for k in range(1, 32):
        cm[k, 0:64, 0:64] = Cr[:, :, k].T
        cm[k, 64:128, 0:64] = -Ci[:, :, k].T
        cm[k, 0:64, 64:128] = Ci[:, :, k].T
        cm[k, 64:128, 64:128] = Cr[:, :, k].T
    return cm.astype(np.float16)


def _build_xk(x):
    """x: [2, 2048, 4096] float32 -> (XKf [NKT, 128, T] fp32, s_x [NKT, 1, T]).

    Hybrid input: kt slots 0,1 of each core ship as exact fp16 (s_x = 1);
    slots 2,3 ship as per-token int8 (SWDGE cast-DMA on the device). The two
    paths ride different DMA rings concurrently.
    """
    xb = np.asarray(x, np.float32).reshape(_T, _NBLK, _B)
    fx = np.fft.rfft(xb, axis=-1)            # [T, I, 33] complex64
    R = fx.real.transpose(2, 1, 0)           # [33, I, T]
    Im = fx.imag.transpose(2, 1, 0)
    XKf = np.empty((_NKT, 128, _T), np.float32)
    XKf[0, 0:64] = R[0]
    XKf[0, 64:128] = R[32]
    XKf[1:32, 0:64] = R[1:32]
    XKf[1:32, 64:128] = Im[1:32]
    absmax = np.abs(XKf).max(axis=1, keepdims=True)        # [NKT, 1, T]
    s_x = np.where(absmax > 0, absmax / 127.0, 1.0).astype(np.float32)
    s_x[(np.arange(_NKT) % _KTC) == 0] = 1.0   # slot 0 ships exact fp16
    return XKf, s_x


def _unpack_y(YKf, bias):
    """YKf: [NKT, 128, T] float32 (already unscaled) -> y [2, 2048, 4096]."""
    re = np.zeros((_NK, _NBLK, _T), np.float32)
    im = np.zeros((_NK, _NBLK, _T), np.float32)
    re[0] = YKf[0, 0:64]
    re[32] = YKf[0, 64:128]
    re[1:32] = YKf[1:32, 0:64]
    im[1:32] = YKf[1:32, 64:128]
    Yf = (re + 1j * im).transpose(2, 1, 0)   # [T, J, 33]
    yb = np.fft.irfft(Yf, n=_B, axis=-1).astype(np.float32)  # [T, J, B]
    y = yb.reshape(_T, _F) + np.asarray(bias, np.float32)
    return np.ascontiguousarray(y.reshape(2, _T // 2, _F))


# v4 feature flags
_ONCHIP_CAST = False   # probe: only ACT casts int8->fp16 fast (3.7us/tile);
                       # GpSimd/DVE are 4x slower. Keep the SWDGE cast-DMA.
_BF16_PSUM = False     # bass asserts matmul output must be fp32
_MM_N = 512            # one PSUM bank per matmul output (512 fp32)


def _build_device():
    import concourse.bacc as bacc
    import concourse.mybir as mybir
    import concourse.tile as tile

    f32 = mybir.dt.float32
    f16 = mybir.dt.float16
    bf16 = mybir.dt.bfloat16
    i8 = mybir.dt.int8
    nc = bacc.Bacc("TRN2", target_bir_lowering=False, debug=False)
    # Slot 0 ships exact fp16 on the sync HWDGE ring (starts the
    # MM/copy/store pipeline ~8us earlier); slots 1-3 are int8 SWDGE
    # cast-DMAs (one fewer cast shortens the emission-paced stream ~3us).
    xf = nc.dram_tensor("xf", [128, _T], f16, kind="ExternalInput")
    xq = nc.dram_tensor("xq", [_KTC - 1, 128, _T], i8, kind="ExternalInput")
    cm = nc.dram_tensor("cm", [128, _KTC * 128], f16, kind="ExternalInput")
    yk = nc.dram_tensor("yk", [_KTC, 2, 128, _T // 2], bf16, kind="ExternalOutput")

    psdt = bf16 if _BF16_PSUM else f32
    ps_cols = 1024                           # 2 PSUM banks (fp32)

    with tile.TileContext(nc) as tc:
        with (
            tc.tile_pool(name="cpool", bufs=1) as cpool,
            tc.tile_pool(name="xpool", bufs=1) as xpool,
            tc.tile_pool(name="ypool", bufs=1) as ypool,
            tc.tile_pool(name="pp", bufs=4, space="PSUM") as pp,
        ):
            xts = []
            xt0 = xpool.tile([128, _T], f16, tag="x0", name="x0")
            xts.append(xt0)
            for g in range(1, _KTC):
                xt = xpool.tile([128, _T], f16, tag=f"x{g}", name=f"x{g}")
                # single_packet: engines drain a whole cast without yielding
                # to the HWDGE store ring at packet boundaries
                nc.gpsimd.dma_start(out=xt[:], in_=xq[g - 1], single_packet=True)
                xts.append(xt)
            cmt = cpool.tile([128, _KTC * 128], f16, tag="cw", name="cw")
            nc.sync.dma_start(out=cmt[:], in_=cm[:])
            nc.sync.dma_start(out=xt0[:], in_=xf[:])

            n_ps = _T // ps_cols             # PSUM tiles per kt
            deferred = []                    # last tile's stores, emitted after
            with nc.allow_low_precision(
                "acc is integer-exact and fits bf16; single-shot matmuls"
            ) if _BF16_PSUM else _noop_ctx():
                for g in range(_KTC):
                    yt = ypool.tile([128, _T], bf16, tag=f"y{g}", name=f"y{g}")
                    for h in range(n_ps):
                        ps = pp.tile([128, ps_cols], psdt)
                        for jj in range(ps_cols // _MM_N):
                            col = h * ps_cols + jj * _MM_N
                            nc.tensor.matmul(
                                ps[:, jj * _MM_N:(jj + 1) * _MM_N],
                                lhsT=cmt[:, g * 128:(g + 1) * 128],
                                rhs=xts[g][:, col:col + _MM_N],
                                start=True,
                                stop=True,
                            )
                        yslice = yt[:, h * ps_cols:(h + 1) * ps_cols]
                        # both copy lanes work the same kt concurrently so a
                        # tile's copies finish in ~1.2us wall, not staggered
                        if h % 2 == 1:
                            nc.scalar.copy(yslice, ps[:])
                        else:
                            nc.vector.tensor_copy(yslice, ps[:])
                        # store each quarter as soon as its copy lands —
                        # HWDGE descriptors are free, and draining the store
                        # backlog during the emission-paced cast phase is
                        # what the idle engine capacity is for. SP ring so
                        # the ACT sequencer isn't interrupted between copies.
                        qw = ps_cols // 2
                        dst = yk[g, h // 2, :, (h % 2) * 2 * qw:((h % 2) * 2 + 2) * qw]
                        if g < _KTC - 1:
                            nc.sync.dma_start(out=dst, in_=yslice)
                        else:
                            deferred.append((dst, yslice))
                # last tile's stores dispatch from BOTH HWDGE sequencers in
                # parallel (the ~0.65us/DIRECT2D dispatch serializes the tail
                # otherwise); emitted after the loop so the ACT-ring ones
                # queue behind its final copy, not between copies
                for i, (dst, src) in enumerate(deferred):
                    if i % 2 == 0:
                        nc.sync.dma_start(out=dst, in_=src)
                    else:
                        nc.scalar.dma_start(out=dst, in_=src)
    nc.compile()
    return nc


from contextlib import contextmanager


@contextmanager
def _noop_ctx():
    yield


def _execute(in_maps, **kwargs):
    from concourse.bass_utils import run_bass_kernel_spmd

    if "nc" not in _CACHE:
        _CACHE["nc"] = _build_device()
    return run_bass_kernel_spmd(
        _CACHE["nc"], in_maps, core_ids=list(range(_NCORES)), **kwargs
    )


def _make_in_maps(x, c):
    XKf, s_x = _build_xk(x)
    cmd = _build_cmat(c)          # [NKT, 128, 128] fp16
    maps = []
    for m in range(_NCORES):
        k0 = m * _KTC
        ks = slice(k0, k0 + _KTC)
        cmm = cmd[ks].transpose(1, 0, 2).reshape(128, _KTC * 128)
        xqm = np.rint(XKf[k0 + 1:k0 + _KTC] / s_x[k0 + 1:k0 + _KTC]).astype(np.int8)
        maps.append(
            {
                "xf": np.ascontiguousarray(XKf[k0].astype(np.float16)),
                "xq": np.ascontiguousarray(xqm),         # [KTC-1, 128, T]
                "cm": np.ascontiguousarray(cmm),
            }
        )
    return maps, s_x


def _gather_yk(results, s_x):
    """Per-core yk [KTC, 2, 128, T//2] bf16 -> unscaled [NKT, 128, T] fp32."""
    acc = np.concatenate(
        [
            np.asarray(r["yk"])
            .astype(np.float32)
            .transpose(0, 2, 1, 3)
            .reshape(_KTC, 128, _T)
            for r in results
        ],
        axis=0,
    )
    return acc * s_x


def kernel(x, c, bias, **_kwargs):
    in_maps, s_x = _make_in_maps(x, c)
    bkr = _execute(in_maps)
    return _unpack_y(_gather_yk(bkr.results, s_x), bias)


# revision 43
# speedup vs baseline: 1.1617x; 1.1617x over previous
"""Block-circulant linear layer (y = x @ W^T + bias, W built from 64x64
circulant blocks) on 8 Trainium2 NeuronCores.

Math: per output block j, input block i: y[t,j] = sum_i circ(c[j,i]) @ x[t,i].
Via the convolution theorem this is, for each rfft bin k:
    Yhat[t,j,k] = sum_i Chat[j,i,k] * Xhat[t,i,k]   (complex)
i.e. 33 independent complex [64 x 64] matmuls over the block index, batched
over tokens. The host does the cheap O(T*F*logB) DFTs + layout packing; the
device does the per-frequency real-packed matmuls.

Real/complex packing (per frequency k, contraction over rows r):
    rhs rows r:   [Xr_i (64) ; Xi_i (64)],  cols = tokens
    lhsT[i,    j] =  Cr[j,i]    lhsT[i,    64+j] = Ci[j,i]
    lhsT[64+i, j] = -Ci[j,i]    lhsT[64+i, 64+j] = Cr[j,i]
    out rows:     [Yr_j (64) ; Yi_j (64)]
Bins k=0 and k=32 are purely real (real input DFT), so they share one tile
(kt=0) with a block-diagonal lhsT; kt=1..31 carry bin k = kt.

Sharding: FREQUENCY-sharded — core m owns kt tiles 4m..4m+3 for ALL 4096
tokens. Same x/y traffic as token-sharding but the lhsT weights are not
replicated (128 KB/core instead of 1 MB/core).

Precision: x is quantized host-side to int8 with a per-(kt, token) scale
(columns of the rhs), shipped as int8 over HBM (halves input DMA bytes) and
cast to fp16 by the SWDGE DMA on load. The matmul is fp16 x fp16 with exact
integer-valued rhs products accumulated in fp32 PSUM, so the device result
equals the host-side integer simulation exactly; output is rounded to bf16
and the host multiplies the scales back in. Measured end-to-end rel err
~6.8e-3 (gate: 2e-2).
"""

import numpy as np
import ml_dtypes

_B = 64          # circulant block size
_NBLK = 64       # input/output blocks (4096/64)
_NK = 33         # rfft bins of a 64-point real signal
_NKT = 32        # packed frequency tiles (k0+k32 share tile 0)
_NCORES = 8
_KTC = _NKT // _NCORES   # kt tiles per core (4)
_T = 4096        # tokens = 2*2048
_F = 4096

_CACHE = {}


def _build_cmat(c):
    """c: [J=64, I=64, B=64] float32 -> packed lhsT matrix [128, NKT*128] fp16."""
    fc = np.fft.rfft(np.asarray(c, np.float32), axis=-1)  # [J, I, 33] complex64
    Cr, Ci = fc.real, fc.imag
    cm = np.zeros((_NKT, 128, 128), np.float32)  # [kt, row, col]
    cm[0, 0:64, 0:64] = Cr[:, :, 0].T
    cm[0, 64:128, 64:128] = Cr[:, :, 32].T
    for k in range(1, 32):
        cm[k, 0:64, 0:64] = Cr[:, :, k].T
        cm[k, 64:128, 0:64] = -Ci[:, :, k].T
        cm[k, 0:64, 64:128] = Ci[:, :, k].T
        cm[k, 64:128, 64:128] = Cr[:, :, k].T
    return cm.astype(np.float16)


def _build_xk(x):
    """x: [2, 2048, 4096] float32 -> (XKf [NKT, 128, T] fp32, s_x [NKT, 1, T]).

    Hybrid input: kt slots 0,1 of each core ship as exact fp16 (s_x = 1);
    slots 2,3 ship as per-token int8 (SWDGE cast-DMA on the device). The two
    paths ride different DMA rings concurrently.
    """
    xb = np.asarray(x, np.float32).reshape(_T, _NBLK, _B)
    fx = np.fft.rfft(xb, axis=-1)            # [T, I, 33] complex64
    R = fx.real.transpose(2, 1, 0)           # [33, I, T]
    Im = fx.imag.transpose(2, 1, 0)
    XKf = np.empty((_NKT, 128, _T), np.float32)
    XKf[0, 0:64] = R[0]
    XKf[0, 64:128] = R[32]
    XKf[1:32, 0:64] = R[1:32]
    XKf[1:32, 64:128] = Im[1:32]
    absmax = np.abs(XKf).max(axis=1, keepdims=True)        # [NKT, 1, T]
    s_x = np.where(absmax > 0, absmax / 127.0, 1.0).astype(np.float32)
    return XKf, s_x


def _unpack_y(YKf, bias):
    """YKf: [NKT, 128, T] float32 (already unscaled) -> y [2, 2048, 4096]."""
    re = np.zeros((_NK, _NBLK, _T), np.float32)
    im = np.zeros((_NK, _NBLK, _T), np.float32)
    re[0] = YKf[0, 0:64]
    re[32] = YKf[0, 64:128]
    re[1:32] = YKf[1:32, 0:64]
    im[1:32] = YKf[1:32, 64:128]
    Yf = (re + 1j * im).transpose(2, 1, 0)   # [T, J, 33]
    yb = np.fft.irfft(Yf, n=_B, axis=-1).astype(np.float32)  # [T, J, B]
    y = yb.reshape(_T, _F) + np.asarray(bias, np.float32)
    return np.ascontiguousarray(y.reshape(2, _T // 2, _F))


# v4 feature flags
_ONCHIP_CAST = False   # probe: only ACT casts int8->fp16 fast (3.7us/tile);
                       # GpSimd/DVE are 4x slower. Keep the SWDGE cast-DMA.
_BF16_PSUM = False     # bass asserts matmul output must be fp32
_MM_N = 512            # one PSUM bank per matmul output (512 fp32)


def _build_device():
    import concourse.bacc as bacc
    import concourse.mybir as mybir
    import concourse.tile as tile

    f32 = mybir.dt.float32
    f16 = mybir.dt.float16
    bf16 = mybir.dt.bfloat16
    i8 = mybir.dt.int8
    nc = bacc.Bacc("TRN2", target_bir_lowering=False, debug=False)
    # All-int8 input via SWDGE cast-DMAs, one per kt tile. Any HWDGE input
    # traffic (even a single fp16 tile) starves the SWDGE cast stream and
    # regresses the whole kernel — measured twice (38.5us, 40.9us).
    xq = nc.dram_tensor("xq", [_KTC, 128, _T], i8, kind="ExternalInput")
    cm = nc.dram_tensor("cm", [128, _KTC * 128], f16, kind="ExternalInput")
    yk = nc.dram_tensor("yk", [_KTC, 2, 128, _T // 2], bf16, kind="ExternalOutput")

    psdt = bf16 if _BF16_PSUM else f32
    ps_cols = 1024                           # 2 PSUM banks (fp32)

    with tile.TileContext(nc) as tc:
        with (
            tc.tile_pool(name="cpool", bufs=1) as cpool,
            tc.tile_pool(name="xpool", bufs=1) as xpool,
            tc.tile_pool(name="ypool", bufs=1) as ypool,
            tc.tile_pool(name="pp", bufs=4, space="PSUM") as pp,
        ):
            xts = []
            for g in range(_KTC):
                xt = xpool.tile([128, _T], f16, tag=f"x{g}", name=f"x{g}")
                # single_packet: engines drain a whole cast without yielding
                # to the HWDGE store ring at packet boundaries
                nc.gpsimd.dma_start(out=xt[:], in_=xq[g], single_packet=True)
                xts.append(xt)
            cmt = cpool.tile([128, _KTC * 128], f16, tag="cw", name="cw")
            nc.sync.dma_start(out=cmt[:], in_=cm[:])

            n_ps = _T // ps_cols             # PSUM tiles per kt
            with nc.allow_low_precision(
                "acc is integer-exact and fits bf16; single-shot matmuls"
            ) if _BF16_PSUM else _noop_ctx():
                for g in range(_KTC):
                    yt = ypool.tile([128, _T], bf16, tag=f"y{g}", name=f"y{g}")
                    for h in range(n_ps):
                        ps = pp.tile([128, ps_cols], psdt)
                        for jj in range(ps_cols // _MM_N):
                            col = h * ps_cols + jj * _MM_N
                            nc.tensor.matmul(
                                ps[:, jj * _MM_N:(jj + 1) * _MM_N],
                                lhsT=cmt[:, g * 128:(g + 1) * 128],
                                rhs=xts[g][:, col:col + _MM_N],
                                start=True,
                                stop=True,
                            )
                        yslice = yt[:, h * ps_cols:(h + 1) * ps_cols]
                        # both copy lanes work the same kt concurrently so a
                        # tile's copies finish in ~1.2us wall, not staggered
                        if h % 2 == 1:
                            nc.scalar.copy(yslice, ps[:])
                        else:
                            nc.vector.tensor_copy(yslice, ps[:])
                        # store each quarter as soon as its copy lands —
                        # HWDGE descriptors are free, and draining the store
                        # backlog during the emission-paced cast phase is
                        # what the idle engine capacity is for. SP ring so
                        # the ACT sequencer isn't interrupted between copies.
                        qw = ps_cols // 2
                        nc.sync.dma_start(
                            out=yk[g, h // 2, :, (h % 2) * 2 * qw:((h % 2) * 2 + 2) * qw],
                            in_=yslice,
                        )
    nc.compile()
    return nc


from contextlib import contextmanager


@contextmanager
def _noop_ctx():
    yield


def _execute(in_maps, **kwargs):
    from concourse.bass_utils import run_bass_kernel_spmd

    if "nc" not in _CACHE:
        _CACHE["nc"] = _build_device()
    return run_bass_kernel_spmd(
        _CACHE["nc"], in_maps, core_ids=list(range(_NCORES)), **kwargs
    )


def _make_in_maps(x, c):
    XKf, s_x = _build_xk(x)
    cmd = _build_cmat(c)          # [NKT, 128, 128] fp16
    maps = []
    for m in range(_NCORES):
        ks = slice(m * _KTC, (m + 1) * _KTC)
        cmm = cmd[ks].transpose(1, 0, 2).reshape(128, _KTC * 128)
        xqm = np.rint(XKf[ks] / s_x[ks]).astype(np.int8)
        maps.append(
            {
                "xq": np.ascontiguousarray(xqm),         # [KTC, 128, T]
                "cm": np.ascontiguousarray(cmm),
            }
        )
    return maps, s_x


def _gather_yk(results, s_x):
    """Per-core yk [KTC, 2, 128, T//2] bf16 -> unscaled [NKT, 128, T] fp32."""
    acc = np.concatenate(
        [
            np.asarray(r["yk"])
            .astype(np.float32)
            .transpose(0, 2, 1, 3)
            .reshape(_KTC, 128, _T)
            for r in results
        ],
        axis=0,
    )
    return acc * s_x


def kernel(x, c, bias, **_kwargs):
    in_maps, s_x = _make_in_maps(x, c)
    bkr = _execute(in_maps)
    return _unpack_y(_gather_yk(bkr.results, s_x), bias)


# revision 48
# speedup vs baseline: 1.2190x; 1.0493x over previous
"""Block-circulant linear layer (y = x @ W^T + bias, W built from 64x64
circulant blocks) on 8 Trainium2 NeuronCores.

Math: per output block j, input block i: y[t,j] = sum_i circ(c[j,i]) @ x[t,i].
Via the convolution theorem this is, for each rfft bin k:
    Yhat[t,j,k] = sum_i Chat[j,i,k] * Xhat[t,i,k]   (complex)
i.e. 33 independent complex [64 x 64] matmuls over the block index, batched
over tokens. The host does the cheap O(T*F*logB) DFTs + layout packing; the
device does the per-frequency real-packed matmuls.

Real/complex packing (per frequency k, contraction over rows r):
    rhs rows r:   [Xr_i (64) ; Xi_i (64)],  cols = tokens
    lhsT[i,    j] =  Cr[j,i]    lhsT[i,    64+j] = Ci[j,i]
    lhsT[64+i, j] = -Ci[j,i]    lhsT[64+i, 64+j] = Cr[j,i]
    out rows:     [Yr_j (64) ; Yi_j (64)]
Bins k=0 and k=32 are purely real (real input DFT), so they share one tile
(kt=0) with a block-diagonal lhsT; kt=1..31 carry bin k = kt.

Sharding: FREQUENCY-sharded — core m owns kt tiles 4m..4m+3 for ALL 4096
tokens. Same x/y traffic as token-sharding but the lhsT weights are not
replicated (128 KB/core instead of 1 MB/core).

Precision: x is quantized host-side to int8 with a per-(kt, token) scale
(columns of the rhs), shipped as int8 over HBM (halves input DMA bytes) and
cast to fp16 by the SWDGE DMA on load. The matmul is fp16 x fp16 with exact
integer-valued rhs products accumulated in fp32 PSUM, so the device result
equals the host-side integer simulation exactly; output is rounded to bf16
and the host multiplies the scales back in. Measured end-to-end rel err
~6.8e-3 (gate: 2e-2).
"""

import numpy as np
import ml_dtypes

_B = 64          # circulant block size
_NBLK = 64       # input/output blocks (4096/64)
_NK = 33         # rfft bins of a 64-point real signal
_NKT = 32        # packed frequency tiles (k0+k32 share tile 0)
_NCORES = 8
_KTC = _NKT // _NCORES   # kt tiles per core (4)
_T = 4096        # tokens = 2*2048
_F = 4096

_CACHE = {}


def _build_cmat(c):
    """c: [J=64, I=64, B=64] float32 -> packed lhsT matrix [128, NKT*128] fp16."""
    fc = np.fft.rfft(np.asarray(c, np.float32), axis=-1)  # [J, I, 33] complex64
    Cr, Ci = fc.real, fc.imag
    cm = np.zeros((_NKT, 128, 128), np.float32)  # [kt, row, col]
    cm[0, 0:64, 0:64] = Cr[:, :, 0].T
    cm[0, 64:128, 64:128] = Cr[:, :, 32].T
    for k in range(1, 32):
        cm[k, 0:64, 0:64] = Cr[:, :, k].T
        cm[k, 64:128, 0:64] = -Ci[:, :, k].T
        cm[k, 0:64, 64:128] = Ci[:, :, k].T
        cm[k, 64:128, 64:128] = Cr[:, :, k].T
    return cm.astype(np.float16)


def _build_xk(x):
    """x: [2, 2048, 4096] float32 -> (XKf [NKT, 128, T] fp32, s_x [NKT, 1, T]).

    Hybrid input: kt slots 0,1 of each core ship as exact fp16 (s_x = 1);
    slots 2,3 ship as per-token int8 (SWDGE cast-DMA on the device). The two
    paths ride different DMA rings concurrently.
    """
    xb = np.asarray(x, np.float32).reshape(_T, _NBLK, _B)
    fx = np.fft.rfft(xb, axis=-1)            # [T, I, 33] complex64
    R = fx.real.transpose(2, 1, 0)           # [33, I, T]
    Im = fx.imag.transpose(2, 1, 0)
    XKf = np.empty((_NKT, 128, _T), np.float32)
    XKf[0, 0:64] = R[0]
    XKf[0, 64:128] = R[32]
    XKf[1:32, 0:64] = R[1:32]
    XKf[1:32, 64:128] = Im[1:32]
    absmax = np.abs(XKf).max(axis=1, keepdims=True)        # [NKT, 1, T]
    s_x = np.where(absmax > 0, absmax / 127.0, 1.0).astype(np.float32)
    return XKf, s_x


def _unpack_y(YKf, bias):
    """YKf: [NKT, 128, T] float32 (already unscaled) -> y [2, 2048, 4096]."""
    re = np.zeros((_NK, _NBLK, _T), np.float32)
    im = np.zeros((_NK, _NBLK, _T), np.float32)
    re[0] = YKf[0, 0:64]
    re[32] = YKf[0, 64:128]
    re[1:32] = YKf[1:32, 0:64]
    im[1:32] = YKf[1:32, 64:128]
    Yf = (re + 1j * im).transpose(2, 1, 0)   # [T, J, 33]
    yb = np.fft.irfft(Yf, n=_B, axis=-1).astype(np.float32)  # [T, J, B]
    y = yb.reshape(_T, _F) + np.asarray(bias, np.float32)
    return np.ascontiguousarray(y.reshape(2, _T // 2, _F))


# v4 feature flags
_ONCHIP_CAST = False   # probe: only ACT casts int8->fp16 fast (3.7us/tile);
                       # GpSimd/DVE are 4x slower. Keep the SWDGE cast-DMA.
_BF16_PSUM = False     # bass asserts matmul output must be fp32
_MM_N = 512            # one PSUM bank per matmul output (512 fp32)


def _build_device():
    import concourse.bacc as bacc
    import concourse.mybir as mybir
    import concourse.tile as tile

    f32 = mybir.dt.float32
    f16 = mybir.dt.float16
    bf16 = mybir.dt.bfloat16
    i8 = mybir.dt.int8
    nc = bacc.Bacc("TRN2", target_bir_lowering=False, debug=False)
    # All-int8 input via SWDGE cast-DMAs (any HWDGE input traffic starves
    # the cast stream — measured twice). The cast datapath is byte-paced
    # (~300 GB/s SBUF-side), so half-tile casts cost nothing extra and let
    # kt0's compute/copies/stores start ~2us earlier.
    xq = nc.dram_tensor("xq", [_KTC, 2, 128, _T // 2], i8, kind="ExternalInput")
    cm = nc.dram_tensor("cm", [128, _KTC * 128], f16, kind="ExternalInput")
    yk = nc.dram_tensor("yk", [_KTC, 2, 128, _T // 2], bf16, kind="ExternalOutput")

    psdt = bf16 if _BF16_PSUM else f32
    ps_cols = 1024                           # 2 PSUM banks (fp32)

    with tile.TileContext(nc) as tc:
        with (
            tc.tile_pool(name="cpool", bufs=1) as cpool,
            tc.tile_pool(name="xpool", bufs=1) as xpool,
            tc.tile_pool(name="ypool", bufs=1) as ypool,
            tc.tile_pool(name="pp", bufs=4, space="PSUM") as pp,
        ):
            xts = []
            hw = _T // 2
            for g in range(_KTC):
                xt = xpool.tile([128, _T], f16, tag=f"x{g}", name=f"x{g}")
                # single_packet: engines drain a whole cast without yielding
                # to the HWDGE store ring at packet boundaries
                nc.gpsimd.dma_start(out=xt[:, 0:hw], in_=xq[g, 0], single_packet=True)
                nc.gpsimd.dma_start(out=xt[:, hw:_T], in_=xq[g, 1], single_packet=True)
                xts.append(xt)
            cmt = cpool.tile([128, _KTC * 128], f16, tag="cw", name="cw")
            nc.sync.dma_start(out=cmt[:], in_=cm[:])

            n_ps = _T // ps_cols             # PSUM tiles per kt
            deferred = []                    # last tile's stores, emitted after
            with nc.allow_low_precision(
                "acc is integer-exact and fits bf16; single-shot matmuls"
            ) if _BF16_PSUM else _noop_ctx():
                for g in range(_KTC):
                    yt = ypool.tile([128, _T], bf16, tag=f"y{g}", name=f"y{g}")
                    for h in range(n_ps):
                        ps = pp.tile([128, ps_cols], psdt)
                        for jj in range(ps_cols // _MM_N):
                            col = h * ps_cols + jj * _MM_N
                            nc.tensor.matmul(
                                ps[:, jj * _MM_N:(jj + 1) * _MM_N],
                                lhsT=cmt[:, g * 128:(g + 1) * 128],
                                rhs=xts[g][:, col:col + _MM_N],
                                start=True,
                                stop=True,
                            )
                        yslice = yt[:, h * ps_cols:(h + 1) * ps_cols]
                        # both copy lanes work the same kt concurrently so a
                        # tile's copies finish in ~1.2us wall, not staggered
                        if h % 2 == 1:
                            nc.scalar.copy(yslice, ps[:])
                        else:
                            nc.vector.tensor_copy(yslice, ps[:])
                        # store each quarter as soon as its copy lands —
                        # HWDGE descriptors are free, and draining the store
                        # backlog during the emission-paced cast phase is
                        # what the idle engine capacity is for. SP ring so
                        # the ACT sequencer isn't interrupted between copies.
                        qw = ps_cols // 2
                        dst = yk[g, h // 2, :, (h % 2) * 2 * qw:((h % 2) * 2 + 2) * qw]
                        if g < _KTC - 1:
                            nc.sync.dma_start(out=dst, in_=yslice)
                        else:
                            deferred.append((dst, yslice))
                # the last tile's stores dispatch from BOTH HWDGE sequencers
                # in parallel (~0.65us per DIRECT2D dispatch serializes the
                # tail on one sequencer); emitted post-loop so the ACT-ring
                # ones queue behind its final copy, not between copies.
                # Store-side HWDGE is safe: dep-gating keeps it off the cast
                # stream's head (unlike HWDGE *input* traffic).
                for i, (dst, src) in enumerate(deferred):
                    if i % 2 == 0:
                        nc.sync.dma_start(out=dst, in_=src)
                    else:
                        nc.scalar.dma_start(out=dst, in_=src)
    nc.compile()
    return nc


from contextlib import contextmanager


@contextmanager
def _noop_ctx():
    yield


def _execute(in_maps, **kwargs):
    from concourse.bass_utils import run_bass_kernel_spmd

    if "nc" not in _CACHE:
        _CACHE["nc"] = _build_device()
    return run_bass_kernel_spmd(
        _CACHE["nc"], in_maps, core_ids=list(range(_NCORES)), **kwargs
    )


def _make_in_maps(x, c):
    XKf, s_x = _build_xk(x)
    cmd = _build_cmat(c)          # [NKT, 128, 128] fp16
    maps = []
    for m in range(_NCORES):
        ks = slice(m * _KTC, (m + 1) * _KTC)
        cmm = cmd[ks].transpose(1, 0, 2).reshape(128, _KTC * 128)
        xqm = (
            np.rint(XKf[ks] / s_x[ks])
            .astype(np.int8)
            .reshape(_KTC, 128, 2, _T // 2)
            .transpose(0, 2, 1, 3)           # [KTC, 2, 128, T//2]
        )
        maps.append(
            {
                "xq": np.ascontiguousarray(xqm),
                "cm": np.ascontiguousarray(cmm),
            }
        )
    return maps, s_x


def _gather_yk(results, s_x):
    """Per-core yk [KTC, 2, 128, T//2] bf16 -> unscaled [NKT, 128, T] fp32."""
    acc = np.concatenate(
        [
            np.asarray(r["yk"])
            .astype(np.float32)
            .transpose(0, 2, 1, 3)
            .reshape(_KTC, 128, _T)
            for r in results
        ],
        axis=0,
    )
    return acc * s_x


def kernel(x, c, bias, **_kwargs):
    in_maps, s_x = _make_in_maps(x, c)
    bkr = _execute(in_maps)
    return _unpack_y(_gather_yk(bkr.results, s_x), bias)


# revision 49
# speedup vs baseline: 1.2271x; 1.0067x over previous
"""Block-circulant linear layer (y = x @ W^T + bias, W built from 64x64
circulant blocks) on 8 Trainium2 NeuronCores.

Math: per output block j, input block i: y[t,j] = sum_i circ(c[j,i]) @ x[t,i].
Via the convolution theorem this is, for each rfft bin k:
    Yhat[t,j,k] = sum_i Chat[j,i,k] * Xhat[t,i,k]   (complex)
i.e. 33 independent complex [64 x 64] matmuls over the block index, batched
over tokens. The host does the cheap O(T*F*logB) DFTs + layout packing; the
device does the per-frequency real-packed matmuls.

Real/complex packing (per frequency k, contraction over rows r):
    rhs rows r:   [Xr_i (64) ; Xi_i (64)],  cols = tokens
    lhsT[i,    j] =  Cr[j,i]    lhsT[i,    64+j] = Ci[j,i]
    lhsT[64+i, j] = -Ci[j,i]    lhsT[64+i, 64+j] = Cr[j,i]
    out rows:     [Yr_j (64) ; Yi_j (64)]
Bins k=0 and k=32 are purely real (real input DFT), so they share one tile
(kt=0) with a block-diagonal lhsT; kt=1..31 carry bin k = kt.

Sharding: FREQUENCY-sharded — core m owns kt tiles 4m..4m+3 for ALL 4096
tokens. Same x/y traffic as token-sharding but the lhsT weights are not
replicated (128 KB/core instead of 1 MB/core).

Precision: x is quantized host-side to int8 with a per-(kt, token) scale
(columns of the rhs), shipped as int8 over HBM (halves input DMA bytes) and
cast to fp16 by the SWDGE DMA on load. The matmul is fp16 x fp16 with exact
integer-valued rhs products accumulated in fp32 PSUM, so the device result
equals the host-side integer simulation exactly; output is rounded to bf16
and the host multiplies the scales back in. Measured end-to-end rel err
~6.8e-3 (gate: 2e-2).
"""

import numpy as np
import ml_dtypes

_B = 64          # circulant block size
_NBLK = 64       # input/output blocks (4096/64)
_NK = 33         # rfft bins of a 64-point real signal
_NKT = 32        # packed frequency tiles (k0+k32 share tile 0)
_NCORES = 8
_KTC = _NKT // _NCORES   # kt tiles per core (4)
_T = 4096        # tokens = 2*2048
_F = 4096

_CACHE = {}


def _build_cmat(c):
    """c: [J=64, I=64, B=64] float32 -> packed lhsT matrix [128, NKT*128] fp16."""
    fc = np.fft.rfft(np.asarray(c, np.float32), axis=-1)  # [J, I, 33] complex64
    Cr, Ci = fc.real, fc.imag
    cm = np.zeros((_NKT, 128, 128), np.float32)  # [kt, row, col]
    cm[0, 0:64, 0:64] = Cr[:, :, 0].T
    cm[0, 64:128, 64:128] = Cr[:, :, 32].T
    for k in range(1, 32):
        cm[k, 0:64, 0:64] = Cr[:, :, k].T
        cm[k, 64:128, 0:64] = -Ci[:, :, k].T
        cm[k, 0:64, 64:128] = Ci[:, :, k].T
        cm[k, 64:128, 64:128] = Cr[:, :, k].T
    return cm.astype(np.float16)


def _build_xk(x):
    """x: [2, 2048, 4096] float32 -> (XKf [NKT, 128, T] fp32, s_x [NKT, 1, T]).

    Hybrid input: kt slots 0,1 of each core ship as exact fp16 (s_x = 1);
    slots 2,3 ship as per-token int8 (SWDGE cast-DMA on the device). The two
    paths ride different DMA rings concurrently.
    """
    xb = np.asarray(x, np.float32).reshape(_T, _NBLK, _B)
    fx = np.fft.rfft(xb, axis=-1)            # [T, I, 33] complex64
    R = fx.real.transpose(2, 1, 0)           # [33, I, T]
    Im = fx.imag.transpose(2, 1, 0)
    XKf = np.empty((_NKT, 128, _T), np.float32)
    XKf[0, 0:64] = R[0]
    XKf[0, 64:128] = R[32]
    XKf[1:32, 0:64] = R[1:32]
    XKf[1:32, 64:128] = Im[1:32]
    absmax = np.abs(XKf).max(axis=1, keepdims=True)        # [NKT, 1, T]
    s_x = np.where(absmax > 0, absmax / 127.0, 1.0).astype(np.float32)
    return XKf, s_x


def _unpack_y(YKf, bias):
    """YKf: [NKT, 128, T] float32 (already unscaled) -> y [2, 2048, 4096]."""
    re = np.zeros((_NK, _NBLK, _T), np.float32)
    im = np.zeros((_NK, _NBLK, _T), np.float32)
    re[0] = YKf[0, 0:64]
    re[32] = YKf[0, 64:128]
    re[1:32] = YKf[1:32, 0:64]
    im[1:32] = YKf[1:32, 64:128]
    Yf = (re + 1j * im).transpose(2, 1, 0)   # [T, J, 33]
    yb = np.fft.irfft(Yf, n=_B, axis=-1).astype(np.float32)  # [T, J, B]
    y = yb.reshape(_T, _F) + np.asarray(bias, np.float32)
    return np.ascontiguousarray(y.reshape(2, _T // 2, _F))


# v4 feature flags
_ONCHIP_CAST = False   # probe: only ACT casts int8->fp16 fast (3.7us/tile);
                       # GpSimd/DVE are 4x slower. Keep the SWDGE cast-DMA.
_BF16_PSUM = False     # bass asserts matmul output must be fp32
_MM_N = 512            # one PSUM bank per matmul output (512 fp32)


def _build_device():
    import concourse.bacc as bacc
    import concourse.mybir as mybir
    import concourse.tile as tile

    f32 = mybir.dt.float32
    f16 = mybir.dt.float16
    bf16 = mybir.dt.bfloat16
    i8 = mybir.dt.int8
    nc = bacc.Bacc("TRN2", target_bir_lowering=False, debug=False)
    # All-int8 input via SWDGE cast-DMAs (any HWDGE input traffic starves
    # the cast stream — measured twice). The cast datapath is byte-paced
    # (~300 GB/s SBUF-side), so half-tile casts cost nothing extra and let
    # kt0's compute/copies/stores start ~2us earlier.
    xq = nc.dram_tensor("xq", [_KTC, 2, 128, _T // 2], i8, kind="ExternalInput")
    cm = nc.dram_tensor("cm", [128, _KTC * 128], f16, kind="ExternalInput")
    yk = nc.dram_tensor("yk", [_KTC, 2, 128, _T // 2], bf16, kind="ExternalOutput")

    psdt = bf16 if _BF16_PSUM else f32
    ps_cols = 1024                           # 2 PSUM banks (fp32)

    with tile.TileContext(nc) as tc:
        with (
            tc.tile_pool(name="cpool", bufs=1) as cpool,
            tc.tile_pool(name="xpool", bufs=1) as xpool,
            tc.tile_pool(name="ypool", bufs=1) as ypool,
            tc.tile_pool(name="pp", bufs=4, space="PSUM") as pp,
        ):
            xts = []
            hw = _T // 2
            qw = _T // 4
            for g in range(_KTC):
                xt = xpool.tile([128, _T], f16, tag=f"x{g}", name=f"x{g}")
                # single_packet: engines drain a whole cast without yielding
                # to the HWDGE store ring at packet boundaries. The first
                # tile loads in quarters so its first matmuls/copies/stores
                # start ~1us earlier (the cast stream ramps slowly at the
                # head while the ACT table load shares the engines).
                if g == 0:
                    for q in range(4):
                        nc.gpsimd.dma_start(
                            out=xt[:, q * qw:(q + 1) * qw],
                            in_=xq[g, q // 2, :, (q % 2) * qw:(q % 2 + 1) * qw],
                            single_packet=True,
                        )
                else:
                    nc.gpsimd.dma_start(
                        out=xt[:, 0:hw], in_=xq[g, 0], single_packet=True
                    )
                    nc.gpsimd.dma_start(
                        out=xt[:, hw:_T], in_=xq[g, 1], single_packet=True
                    )
                xts.append(xt)
            cmt = cpool.tile([128, _KTC * 128], f16, tag="cw", name="cw")
            nc.sync.dma_start(out=cmt[:], in_=cm[:])

            n_ps = _T // ps_cols             # PSUM tiles per kt
            deferred = []                    # last tile's stores, emitted after
            with nc.allow_low_precision(
                "acc is integer-exact and fits bf16; single-shot matmuls"
            ) if _BF16_PSUM else _noop_ctx():
                for g in range(_KTC):
                    yt = ypool.tile([128, _T], bf16, tag=f"y{g}", name=f"y{g}")
                    for h in range(n_ps):
                        ps = pp.tile([128, ps_cols], psdt)
                        for jj in range(ps_cols // _MM_N):
                            col = h * ps_cols + jj * _MM_N
                            nc.tensor.matmul(
                                ps[:, jj * _MM_N:(jj + 1) * _MM_N],
                                lhsT=cmt[:, g * 128:(g + 1) * 128],
                                rhs=xts[g][:, col:col + _MM_N],
                                start=True,
                                stop=True,
                            )
                        yslice = yt[:, h * ps_cols:(h + 1) * ps_cols]
                        # both copy lanes work the same kt concurrently so a
                        # tile's copies finish in ~1.2us wall, not staggered
                        if h % 2 == 1:
                            nc.scalar.copy(yslice, ps[:])
                        else:
                            nc.vector.tensor_copy(yslice, ps[:])
                        # store each quarter as soon as its copy lands —
                        # HWDGE descriptors are free, and draining the store
                        # backlog during the emission-paced cast phase is
                        # what the idle engine capacity is for. SP ring so
                        # the ACT sequencer isn't interrupted between copies.
                        qw = ps_cols // 2
                        dst = yk[g, h // 2, :, (h % 2) * 2 * qw:((h % 2) * 2 + 2) * qw]
                        if g < _KTC - 1:
                            nc.sync.dma_start(out=dst, in_=yslice)
                        else:
                            deferred.append((dst, yslice))
                # the last tile's stores dispatch from BOTH HWDGE sequencers
                # in parallel (~0.65us per DIRECT2D dispatch serializes the
                # tail on one sequencer); emitted post-loop so the ACT-ring
                # ones queue behind its final copy, not between copies.
                # Store-side HWDGE is safe: dep-gating keeps it off the cast
                # stream's head (unlike HWDGE *input* traffic).
                for i, (dst, src) in enumerate(deferred):
                    if i % 2 == 0:
                        nc.sync.dma_start(out=dst, in_=src)
                    else:
                        nc.scalar.dma_start(out=dst, in_=src)
    nc.compile()
    return nc


from contextlib import contextmanager


@contextmanager
def _noop_ctx():
    yield


def _execute(in_maps, **kwargs):
    from concourse.bass_utils import run_bass_kernel_spmd

    if "nc" not in _CACHE:
        _CACHE["nc"] = _build_device()
    return run_bass_kernel_spmd(
        _CACHE["nc"], in_maps, core_ids=list(range(_NCORES)), **kwargs
    )


def _make_in_maps(x, c):
    XKf, s_x = _build_xk(x)
    cmd = _build_cmat(c)          # [NKT, 128, 128] fp16
    maps = []
    for m in range(_NCORES):
        ks = slice(m * _KTC, (m + 1) * _KTC)
        cmm = cmd[ks].transpose(1, 0, 2).reshape(128, _KTC * 128)
        xqm = (
            np.rint(XKf[ks] / s_x[ks])
            .astype(np.int8)
            .reshape(_KTC, 128, 2, _T // 2)
            .transpose(0, 2, 1, 3)           # [KTC, 2, 128, T//2]
        )
        maps.append(
            {
                "xq": np.ascontiguousarray(xqm),
                "cm": np.ascontiguousarray(cmm),
            }
        )
    return maps, s_x


def _gather_yk(results, s_x):
    """Per-core yk [KTC, 2, 128, T//2] bf16 -> unscaled [NKT, 128, T] fp32."""
    acc = np.concatenate(
        [
            np.asarray(r["yk"])
            .astype(np.float32)
            .transpose(0, 2, 1, 3)
            .reshape(_KTC, 128, _T)
            for r in results
        ],
        axis=0,
    )
    return acc * s_x


def kernel(x, c, bias, **_kwargs):
    in_maps, s_x = _make_in_maps(x, c)
    bkr = _execute(in_maps)
    return _unpack_y(_gather_yk(bkr.results, s_x), bias)
